# revision 1
# baseline (speedup 1.0000x reference)
"""ChildSum TreeLSTM cell for 8 Trainium2 NeuronCores — v4.

Sharding: nodes/edges partitioned by edge_dst owner across 8 cores
(25000 nodes each). Nodes are packed into NBLK blocks of 128 destination
slots with a STRICT cap of 128 edges per block (balanced bin packing;
NBLK escalates by 8 if packing fails). Host materializes one uniform
bf16 slab section per block: xT (feat-major), h_childT (feat-major, for
the f-gate matmul), h_tildT (host fp32 segment-sum of h[src], feat-major),
c_child (edge-major) and the one-hot scatter matrix S (edge-major).
Weights replicated per core, bf16; the tanh-column weights of
W_iou/U_iou are pre-doubled so one sigmoid covers i, o, u AND f
(tanh(z) = 2*sigmoid(2z)-1, fixed up on DVE).

Device per block j (one fused PSUM tile [128,1024] = io|u|f, bufs=3):
  PE : f_ps += h_childT.T @ U_f ; io_ps/u_ps += {xT,h_tildT}.T @ WU
       c_agg(j-1) += S.T @ fc     (software-pipelined by one block)
  ACT: sig = sigmoid(io|u|f)      (one op per block, PSUM -> SBUF super)
  DVE: fc = sig_f * c_child ; c_agg copy -> SBUF super
Apply phase batched per super (8 blocks): u=2s-1 (DVE), iu (gpsimd),
c_new = iu + c_agg (DVE), tanh (ACT), h_new = o*tanh (gpsimd),
contiguous [128, 2048] output DMAs on the sync queue.
"""
import os
import sys

for _p in ("/opt/trn_rl_repo",):
    if _p not in sys.path:
        sys.path.insert(0, _p)

import heapq

import numpy as np
import ml_dtypes

import concourse.bass as bass
import concourse.bacc as bacc
import concourse.mybir as mybir
import concourse.tile as tile
from concourse.bass_utils import run_bass_kernel_spmd

f32 = mybir.dt.float32
bf16 = mybir.dt.bfloat16
f8 = mybir.dt.float8e4
nbf16 = ml_dtypes.bfloat16
nf8 = ml_dtypes.float8_e4m3

N_CORES = 8
BLK = 128    # destination nodes (and max edges) per block
SUP = 8      # blocks per super-step (apply-phase batching)
H = 256
X = 256
# xDR 128 (fp8 DoubleRow-packed, io gates) | h_childT 256 (bf16, f gate)
# | htDR 128 (fp8, io gates) | xT 256 (bf16, u gate) | h_tildT 256 (bf16,
# u gate) | c_child 256 (bf16) | S 128 (bf16) -- fp8 bytes live in bf16
# columns, read on-device via bitcast. The u gate stays bf16: fp8 there
# pushes rel err past the tolerance (tanh slope ~1 vs sigmoid ~0.2).
BCOLS = 1408

LAST_EXEC_TIME_NS = None
_PROGRAM_CACHE = {}


def _build_program(nblk, f_bias_zero, iou_bias_zero):
    assert nblk % SUP == 0
    nsup = nblk // SUP
    scols = SUP * BCOLS

    nc = bacc.Bacc(None, target_bir_lowering=False, debug=False)

    slab_d = nc.declare_dram_parameter("slab", [128, nblk * BCOLS], bf16,
                                       isOutput=False)
    ufw_d = nc.declare_dram_parameter("ufw", [128, 2 * H], bf16, isOutput=False)
    # DoubleRow-packed fp8 weights: [k, t(x|htild), j(2), n] as bf16 bytes
    wio_d = nc.declare_dram_parameter("wio", [128, 2 * 512], bf16,
                                      isOutput=False)
    wuu_d = nc.declare_dram_parameter("wuu", [128, 4 * 256], bf16,
                                      isOutput=False)
    if not f_bias_zero:
        ufb_d = nc.declare_dram_parameter("ufb", [1, H], bf16, isOutput=False)
    if not iou_bias_zero:
        bio_d = nc.declare_dram_parameter("bio", [1, 512], bf16, isOutput=False)
        buu_d = nc.declare_dram_parameter("buu", [1, 256], bf16, isOutput=False)

    hout_d = nc.declare_dram_parameter("houtP", [128, nblk * H], bf16,
                                       isOutput=True)
    cout_d = nc.declare_dram_parameter("coutP", [128, nblk * H], bf16,
                                       isOutput=True)

    SIG = mybir.ActivationFunctionType.Sigmoid
    TANH = mybir.ActivationFunctionType.Tanh
    MULT = mybir.AluOpType.mult
    ADD = mybir.AluOpType.add

    with tile.TileContext(nc) as tc:
        with (
            tc.tile_pool(name="const", bufs=1) as cpool,
            tc.tile_pool(name="io", bufs=3) as iop,
            tc.tile_pool(name="work", bufs=3) as wp,
            tc.tile_pool(name="sup", bufs=2) as sp,
            tc.tile_pool(name="sup4", bufs=4) as sp4,
            tc.tile_pool(name="ps_blk", bufs=3, space="PSUM") as psb,
            tc.tile_pool(name="ps_cg", bufs=2, space="PSUM") as psc,
        ):
            ufw_t = cpool.tile([128, 2 * H], bf16, tag="ufw", name="ufw")
            nc.sync.dma_start(out=ufw_t[:], in_=ufw_d[:])
            wio_t = cpool.tile([128, 2 * 512], bf16, tag="wio", name="wio")
            nc.sync.dma_start(out=wio_t[:], in_=wio_d[:])
            wuu_t = cpool.tile([128, 4 * 256], bf16, tag="wuu", name="wuu")
            nc.sync.dma_start(out=wuu_t[:], in_=wuu_d[:])
            wio8 = [wio_t[:, t * 512:(t + 1) * 512].bitcast(f8).rearrange(
                "p (j n) -> p j n", j=2) for t in range(2)]
            if not f_bias_zero or not iou_bias_zero:
                ones_t = cpool.tile([1, 128], bf16)
                nc.vector.memset(ones_t[:], 1.0)
            if not f_bias_zero:
                ufb_t = cpool.tile([1, H], bf16)
                nc.sync.dma_start(out=ufb_t[:], in_=ufb_d[:])
            if not iou_bias_zero:
                bio_t = cpool.tile([1, 512], bf16)
                nc.sync.dma_start(out=bio_t[:], in_=bio_d[:])
                buu_t = cpool.tile([1, 256], bf16)
                nc.sync.dma_start(out=buu_t[:], in_=buu_d[:])

            state = {}  # rolling per-super tiles + pending cagg work

            def emit_cagg(pend):
                jj, fc_t, s_ap = pend
                b = jj % SUP
                cg_t = psc.tile([128, H], f32, tag="cg", name="cg")
                nc.tensor.matmul(out=cg_t[:], lhsT=s_ap, rhs=fc_t[:],
                                 start=True, stop=True)
                nc.vector.tensor_copy(out=state[("cgs", jj // SUP)][:, b, :],
                                      in_=cg_t[:])

            def emit_apply_piece(s, piece):
                # fuio is section-major [128, sect(i,o,u,f), SUP, 256] so every
                # apply slice below is contiguous (keeps DVE packed fast path).
                # Pieces are spread across blocks so no engine FIFO gets a
                # burst that would stall the PE's software pipeline.
                fuio = state[("fuio", s)]
                if piece == 0:
                    u2 = sp.tile([128, SUP, H], bf16, tag="u2", name="u2")
                    nc.vector.tensor_scalar(
                        out=u2[:], in0=fuio[:, 2, :, :],
                        scalar1=2.0, scalar2=-1.0, op0=MULT, op1=ADD)
                    state[("u2", s)] = u2
                elif piece == 1:
                    iu = sp.tile([128, SUP, H], bf16, tag="iu", name="iu")
                    nc.vector.tensor_tensor(out=iu[:], in0=fuio[:, 0, :, :],
                                            in1=state[("u2", s)][:], op=MULT)
                    state[("iu", s)] = iu
                elif piece == 2:
                    cn = sp.tile([128, SUP, H], bf16, tag="cn", name="cn")
                    nc.vector.tensor_tensor(out=cn[:], in0=state[("iu", s)][:],
                                            in1=state[("cgs", s)][:], op=ADD)
                    state[("cn", s)] = cn
                    nc.sync.dma_start(
                        out=cout_d[:, s * SUP * H:(s + 1) * SUP * H],
                        in_=cn[:])
                elif piece == 3:
                    th = sp.tile([128, SUP, H], bf16, tag="th", name="th")
                    nc.scalar.activation(out=th[:], in_=state[("cn", s)][:],
                                         func=TANH)
                    state[("th", s)] = th
                else:
                    hn = sp.tile([128, SUP, H], bf16, tag="hn", name="hn")
                    nc.vector.tensor_tensor(out=hn[:], in0=fuio[:, 1, :, :],
                                            in1=state[("th", s)][:], op=MULT)
                    nc.sync.dma_start(
                        out=hout_d[:, s * SUP * H:(s + 1) * SUP * H],
                        in_=hn[:])

            pending = []
            for j in range(nblk):
                s, b = j // SUP, j % SUP
                if b == 0:
                    slab_t = iop.tile([128, scols], bf16, tag="slab",
                                      name="slab")
                    nc.sync.dma_start(
                        out=slab_t[:],
                        in_=slab_d[:, s * scols:(s + 1) * scols])
                    state[("slab", s)] = slab_t
                    state[("fuio", s)] = sp4.tile([128, 4, SUP, H], bf16,
                                                  tag="fuio", name="fuio")
                    state[("cgs", s)] = sp4.tile([128, SUP, H], bf16,
                                                 tag="cgs", name="cgs")

                slab_t = state[("slab", s)]
                o = b * BCOLS
                blk = psb.tile([128, 4, H], f32, tag="blk", name="blk")
                # f-gate matmuls (bf16) -> blk[:, 3, :]
                for fi in range(2):
                    nc.tensor.matmul(
                        out=blk[:, 3, :],
                        lhsT=slab_t[:, o + 128 + fi * 128:o + 128 + fi * 128 + 128],
                        rhs=ufw_t[:, fi * H:(fi + 1) * H],
                        start=(fi == 0), stop=(fi == 1 and f_bias_zero))
                if not f_bias_zero:
                    nc.tensor.matmul(out=blk[:, 3, :], lhsT=ones_t[:],
                                     rhs=ufb_t[:], start=False, stop=True)
                # io matmuls (fp8 DoubleRow, K=256 per pass) -> blk[:, 0:2, :]
                for t in range(2):
                    oo = o if t == 0 else o + 384
                    lhsT = slab_t[:, oo:oo + 128].bitcast(f8).rearrange(
                        "p (j m) -> p j m", j=2)
                    nc.tensor.matmul(
                        out=blk[:, 0:2, :], lhsT=lhsT, rhs=wio8[t],
                        start=(t == 0), stop=(t == 1 and iou_bias_zero),
                        perf_mode=mybir.MatmulPerfMode.DoubleRow)
                # u matmuls (bf16) -> blk[:, 2, :]
                for t in range(4):
                    nc.tensor.matmul(
                        out=blk[:, 2, :],
                        lhsT=slab_t[:, o + 512 + t * 128:o + 512 + t * 128 + 128],
                        rhs=wuu_t[:, t * 256:(t + 1) * 256],
                        start=(t == 0), stop=(t == 3 and iou_bias_zero))
                if not iou_bias_zero:
                    nc.tensor.matmul(out=blk[:, 0:2, :], lhsT=ones_t[:],
                                     rhs=bio_t[:], start=False, stop=True)
                    nc.tensor.matmul(out=blk[:, 2, :], lhsT=ones_t[:],
                                     rhs=buu_t[:], start=False, stop=True)
                # c_agg scatter lagged by two blocks slots in here on the PE,
                # giving sigma(j-2)+fc(j-2) two full PE-blocks of slack
                if len(pending) >= 2:
                    emit_cagg(pending.pop(0))
                # sigmoid over i|o|u|f -> SBUF super tile (section-major)
                nc.scalar.activation(
                    out=state[("fuio", s)][:, :, b, :],
                    in_=blk[:], func=SIG)
                # fc = f * c_child
                fc_t = wp.tile([128, H], bf16, tag="fc", name="fc")
                nc.vector.tensor_tensor(
                    out=fc_t[:], in0=state[("fuio", s)][:, 3, b, :],
                    in1=slab_t[:, o + 1024:o + 1280], op=MULT)
                pending.append((j, fc_t, slab_t[:, o + 1280:o + 1408]))
                # apply phase for super s-2, one piece per block so no engine
                # FIFO gets a burst that stalls the PE's software pipeline
                if 1 <= b <= 5 and s >= 2:
                    emit_apply_piece(s - 2, b - 1)

            for p in pending:
                emit_cagg(p)
            # interleave the final two supers' apply pieces so their
            # cross-engine chains overlap instead of running back-to-back
            tail = [nsup - 2, nsup - 1] if nsup >= 2 else [nsup - 1]
            for piece in range(5):
                for s in tail:
                    emit_apply_piece(s, piece)

    nc.compile()
    return nc


def _pack_blocks(deg, nblk):
    """Strict balanced bin packing: nodes (desc degree) into nblk blocks of
    BLK node slots and at most BLK edges. Returns None if impossible."""
    npc = deg.shape[0]
    order = np.argsort(-deg, kind="stable")
    rem_e = np.full(nblk, BLK, np.int64)
    rem_n = np.full(nblk, BLK, np.int64)
    heap = [(-BLK, b) for b in range(nblk)]
    heapq.heapify(heap)
    assign = np.empty(npc, np.int64)
    for nd in order:
        d = int(deg[nd])
        tmp = []
        placed = False
        while heap:
            negre, b = heapq.heappop(heap)
            if rem_n[b] == 0:
                continue
            if d <= rem_e[b]:
                rem_e[b] -= d
                rem_n[b] -= 1
                assign[nd] = b
                placed = True
                if rem_n[b] > 0:
                    heapq.heappush(heap, (-rem_e[b], b))
                break
            tmp.append((negre, b))
        for t in tmp:
            heapq.heappush(heap, t)
        if not placed:
            return None
    blk_sorted = np.argsort(assign[order], kind="stable")
    perm = order[blk_sorted]  # nodes in block-major placement order
    pos = np.empty(npc, np.int64)
    used = np.bincount(assign, minlength=nblk)
    starts = np.concatenate([[0], np.cumsum(used)[:-1]])
    pos[perm] = np.arange(npc) - np.repeat(starts, used)
    return assign, pos, perm, used


def _prep_inputs(x, h, c, W_iou, U_iou, b_iou, U_f_w, U_f_b,
                 edge_src, edge_dst):
    n = x.shape[0]
    assert n % N_CORES == 0
    npc = n // N_CORES

    x32 = np.asarray(x, np.float32)
    hbf = np.asarray(h, np.float32).astype(nbf16)
    h32 = np.asarray(h, np.float32)
    cbf = np.asarray(c, np.float32).astype(nbf16)
    edge_src = np.asarray(edge_src, np.int64)
    edge_dst = np.asarray(edge_dst, np.int64)
    owner = edge_dst // npc

    nblk = max(-(-npc // BLK), 1)
    nblk = -(-nblk // SUP) * SUP
    packs = None
    while packs is None:
        packs = []
        for kcore in range(N_CORES):
            ldst = edge_dst[owner == kcore] - kcore * npc
            deg = np.bincount(ldst, minlength=npc)
            if deg.max(initial=0) > BLK:
                raise ValueError("node degree exceeds block capacity")
            r = _pack_blocks(deg, nblk)
            if r is None:
                packs = None
                nblk += SUP
                break
            packs.append(r)
    loc = nblk * BLK

    ufwT = np.asarray(U_f_w, np.float32).T          # [in 256, out 256]
    ufw_s = np.ascontiguousarray(
        ufwT.reshape(2, 128, H).transpose(1, 0, 2).reshape(128, 2 * H)
    ).astype(nbf16)
    wiouT = np.asarray(W_iou, np.float32).T          # [256, 768]
    uiouT = np.asarray(U_iou, np.float32).T
    # io weights DoubleRow-packed fp8 [k, t(x|htild), j(2), n]; u weights
    # bf16 [k, t(x0,x1,ht0,ht1), n], pre-doubled for the sigmoid-tanh trick
    wio_s = np.empty((128, 2, 2, 512), np.float32)
    wuu_s = np.empty((128, 4, 256), np.float32)
    for t in range(2):
        srcw = wiouT if t == 0 else uiouT
        for jhalf in range(2):
            rows = srcw[jhalf * 128:(jhalf + 1) * 128]
            wio_s[:, t, jhalf, :] = rows[:, 0:512]
            wuu_s[:, 2 * t + jhalf, :] = rows[:, 512:768] * 2.0
    wio_s = np.ascontiguousarray(
        wio_s.reshape(128, 2 * 2 * 512).astype(nf8)).view(nbf16)
    wuu_s = wuu_s.reshape(128, 4 * 256).astype(nbf16)
    b_iou_f = np.asarray(b_iou, np.float32).reshape(-1)
    U_f_b_f = np.asarray(U_f_b, np.float32).reshape(-1)
    f_bias_zero = not U_f_b_f.any()
    iou_bias_zero = not b_iou_f.any()

    in_maps = []
    perms = []
    for kcore in range(N_CORES):
        m = owner == kcore
        ldst = edge_dst[m] - kcore * npc
        src = edge_src[m]
        assign, pos, perm, used = packs[kcore]
        blk_id = assign[ldst]
        dstrel = pos[ldst]

        # edge slot layout: 128 slots per block, edges packed in order
        cnt = np.bincount(blk_id, minlength=nblk)
        assert cnt.max(initial=0) <= BLK
        start = np.zeros(nblk, np.int64)
        np.cumsum(cnt[:-1], out=start[1:])
        eorder = np.argsort(blk_id, kind="stable")
        slot_in_blk = np.arange(blk_id.size) - start[blk_id[eorder]]
        flat_pos = blk_id[eorder] * BLK + slot_in_blk
        tot_e = nblk * BLK

        gsrc = src[eorder]
        hch = np.zeros((tot_e, H), nbf16)
        cch = np.zeros((tot_e, H), nbf16)
        hch[flat_pos] = hbf[gsrc]
        cch[flat_pos] = cbf[gsrc]
        Sfl = np.zeros((tot_e, BLK), nbf16)
        Sfl[flat_pos, dstrel[eorder]] = 1.0

        gperm = perm + kcore * npc
        cols = np.concatenate([
            np.arange(s0, s0 + u) for s0, u in zip(range(0, loc, BLK), used)])
        xN = np.zeros((loc, X), np.float32)
        xN[cols] = x32[gperm]

        htild = np.zeros((loc, H), np.float32)
        if src.size:
            slot = blk_id * BLK + dstrel
            so = np.argsort(slot, kind="stable")
            slot_s = slot[so]
            bounds = np.concatenate(
                [[0], np.flatnonzero(np.diff(slot_s)) + 1])
            htild[slot_s[bounds]] = np.add.reduceat(
                h32[src[so]], bounds, axis=0)

        # assemble uniform slab: [128, nblk, BCOLS]
        # x / h_tild for the io gates are fp8 DoubleRow-packed [k, j, m]
        # (bytes in bf16 cols); the u gate gets separate bf16 copies
        slab = np.empty((128, nblk, BCOLS), nbf16)
        slab[:, :, 0:128] = np.ascontiguousarray(
            xN.astype(nf8).reshape(nblk, 128, 2, 128).transpose(3, 0, 2, 1)
        ).reshape(128, nblk, 256).view(nbf16)
        slab[:, :, 128:384] = hch.reshape(nblk, 128, 2, 128)\
            .transpose(3, 0, 2, 1).reshape(128, nblk, 256)
        slab[:, :, 384:512] = np.ascontiguousarray(
            htild.astype(nf8).reshape(nblk, 128, 2, 128).transpose(3, 0, 2, 1)
        ).reshape(128, nblk, 256).view(nbf16)
        slab[:, :, 512:768] = xN.astype(nbf16).reshape(nblk, 128, 2, 128)\
            .transpose(3, 0, 2, 1).reshape(128, nblk, 256)
        slab[:, :, 768:1024] = htild.astype(nbf16)\
            .reshape(nblk, 128, 2, 128).transpose(3, 0, 2, 1)\
            .reshape(128, nblk, 256)
        slab[:, :, 1024:1280] = cch.reshape(nblk, 128, 256)\
            .transpose(1, 0, 2)
        slab[:, :, 1280:1408] = Sfl.reshape(nblk, 128, BLK)\
            .transpose(1, 0, 2)
        slab = slab.reshape(128, nblk * BCOLS)

        im = {"slab": slab, "ufw": ufw_s, "wio": wio_s, "wuu": wuu_s}
        if not f_bias_zero:
            im["ufb"] = U_f_b_f.reshape(1, H).astype(nbf16)
        if not iou_bias_zero:
            im["bio"] = b_iou_f[0:512].reshape(1, 512).astype(nbf16)
            im["buu"] = (2.0 * b_iou_f[512:768]).reshape(1, 256).astype(nbf16)
        in_maps.append(im)
        perms.append((gperm, cols))

    meta = dict(n=n, npc=npc, nblk=nblk, loc=loc,
                f_bias_zero=f_bias_zero, iou_bias_zero=iou_bias_zero,
                perms=perms)
    return in_maps, meta


def kernel(x, h, c, W_iou, U_iou, b_iou, U_f_w, U_f_b, edge_src, edge_dst,
           _trace=False):
    global LAST_EXEC_TIME_NS
    in_maps, meta = _prep_inputs(x, h, c, W_iou, U_iou, b_iou, U_f_w, U_f_b,
                                 edge_src, edge_dst)
    key = (meta["nblk"], meta["f_bias_zero"], meta["iou_bias_zero"])
    nc = _PROGRAM_CACHE.get(key)
    if nc is None:
        nc = _build_program(meta["nblk"], meta["f_bias_zero"],
                            meta["iou_bias_zero"])
        _PROGRAM_CACHE[key] = nc
    if not _trace:
        os.environ.setdefault("BASS_NEVER_TRACE", "1")
    res = run_bass_kernel_spmd(nc, in_maps, list(range(N_CORES)),
                               trace=_trace,
                               trace_cores=[0] if _trace else None)
    if _trace:
        LAST_EXEC_TIME_NS = res.exec_time_ns

    n, nblk, loc = meta["n"], meta["nblk"], meta["loc"]
    h_new = np.empty((n, H), np.float32)
    c_new = np.empty((n, H), np.float32)
    for kcore in range(N_CORES):
        gperm, cols = meta["perms"][kcore]
        hP = res.results[kcore]["houtP"].reshape(128, nblk, H)\
            .transpose(1, 0, 2).reshape(loc, H)
        cP = res.results[kcore]["coutP"].reshape(128, nblk, H)\
            .transpose(1, 0, 2).reshape(loc, H)
        h_new[gperm] = hP[cols].astype(np.float32)
        c_new[gperm] = cP[cols].astype(np.float32)
    return h_new, c_new



# revision 2
# speedup vs baseline: 1.0170x; 1.0170x over previous
"""ChildSum TreeLSTM cell for 8 Trainium2 NeuronCores — v5.

Sharding: nodes/edges partitioned by edge_dst owner across 8 cores
(25000 nodes each). Nodes are packed into NBLK blocks of 128 destination
slots with a STRICT cap of 128 edges per block (balanced bin packing;
NBLK escalates by 8 if packing fails). Host materializes one uniform
bf16 slab section per block.

v5 numerics: the u gate moves to fp8 DoubleRow with BOTH a data
residual (dx8 = Q(x - deq(x8))) and a weight residual
(dW8 = Q(W - deq(W8))), so u = (x8+dx8)@W8 + x8@dW8 — error ~delta^2,
matching bf16 quality while shipping x/htild once in fp8 for io AND u
(512B/partition/block saved vs v4). f gate stays bf16 (error budget).
The tanh-column weights are pre-doubled so one sigmoid covers i, o, u
AND f (tanh(z) = 2*sigmoid(2z)-1, fixed up on DVE).

Device per block j (one fused PSUM tile [128,1024] = io|u|f, bufs=3):
  PE : io_ps += {x8,ht8}DR @ Wio8 ; u_ps += 6 DR residual matmuls ;
       f_ps += h_childT @ U_f (bf16) ; c_agg(j-2) += S.T @ fc (lagged)
  ACT: sig = sigmoid(io|u|f)      (one op per block, PSUM -> SBUF super)
  GPS: fc = sig_f * c_child       (SBUF-only, Pool engine)
  DVE: c_agg copy PSUM -> SBUF super
Apply phase batched per super (8 blocks): u=2s-1, iu, cn=iu+c_agg (DVE),
tanh (ACT), h_new = o*tanh (DVE), contiguous [128, 2048] output DMAs.
"""
import os
import sys

for _p in ("/opt/trn_rl_repo",):
    if _p not in sys.path:
        sys.path.insert(0, _p)

import heapq

import numpy as np
import ml_dtypes

import concourse.bass as bass
import concourse.bacc as bacc
import concourse.mybir as mybir
import concourse.tile as tile
from concourse.bass_utils import run_bass_kernel_spmd

f32 = mybir.dt.float32
bf16 = mybir.dt.bfloat16
f8 = mybir.dt.float8e4
nbf16 = ml_dtypes.bfloat16
nf8 = ml_dtypes.float8_e4m3

N_CORES = 8
BLK = 128    # destination nodes (and max edges) per block
SUP = 8      # blocks per super-step (apply-phase batching)
H = 256
X = 256
# x8 128 | dx8 128 | ht8 128 | dht8 128 (all fp8 DoubleRow-packed, bytes
# in bf16 cols) | h_childT 256 (bf16, f gate) | c_child 256 (bf16)
# | S 128 (bf16)
BCOLS = 1152

LAST_EXEC_TIME_NS = None
_PROGRAM_CACHE = {}

DR = mybir.MatmulPerfMode.DoubleRow


def _build_program(nblk, f_bias_zero, iou_bias_zero):
    assert nblk % SUP == 0
    nsup = nblk // SUP
    scols = SUP * BCOLS

    nc = bacc.Bacc(None, target_bir_lowering=False, debug=False)

    slab_d = nc.declare_dram_parameter("slab", [128, nblk * BCOLS], bf16,
                                       isOutput=False)
    ufw_d = nc.declare_dram_parameter("ufw", [128, 2 * H], bf16, isOutput=False)
    # DoubleRow-packed fp8 weights (bytes as bf16 cols):
    # io per input [k, j(2), 512]; u W8 + dW8 per input [k, j(2), 256]
    wxio_d = nc.declare_dram_parameter("wxio", [128, 512], bf16, isOutput=False)
    whio_d = nc.declare_dram_parameter("whio", [128, 512], bf16, isOutput=False)
    wxu_d = nc.declare_dram_parameter("wxu", [128, 256], bf16, isOutput=False)
    dwxu_d = nc.declare_dram_parameter("dwxu", [128, 256], bf16, isOutput=False)
    whu_d = nc.declare_dram_parameter("whu", [128, 256], bf16, isOutput=False)
    dwhu_d = nc.declare_dram_parameter("dwhu", [128, 256], bf16, isOutput=False)
    if not f_bias_zero:
        ufb_d = nc.declare_dram_parameter("ufb", [1, H], bf16, isOutput=False)
    if not iou_bias_zero:
        bio_d = nc.declare_dram_parameter("bio", [1, 512], bf16, isOutput=False)
        buu_d = nc.declare_dram_parameter("buu", [1, 256], bf16, isOutput=False)

    hout_d = nc.declare_dram_parameter("houtP", [128, nblk * H], bf16,
                                       isOutput=True)
    cout_d = nc.declare_dram_parameter("coutP", [128, nblk * H], bf16,
                                       isOutput=True)

    SIG = mybir.ActivationFunctionType.Sigmoid
    TANH = mybir.ActivationFunctionType.Tanh
    MULT = mybir.AluOpType.mult
    ADD = mybir.AluOpType.add

    def dr(ap, n):
        return ap.bitcast(f8).rearrange("p (j n) -> p j n", j=2)

    with tile.TileContext(nc) as tc:
        with (
            tc.tile_pool(name="const", bufs=1) as cpool,
            tc.tile_pool(name="io", bufs=3) as iop,
            tc.tile_pool(name="work", bufs=3) as wp,
            tc.tile_pool(name="sup", bufs=2) as sp,
            tc.tile_pool(name="sup4", bufs=4) as sp4,
            tc.tile_pool(name="ps_blk", bufs=3, space="PSUM") as psb,
            tc.tile_pool(name="ps_cg", bufs=2, space="PSUM") as psc,
        ):
            ufw_t = cpool.tile([128, 2 * H], bf16, tag="ufw", name="ufw")
            nc.sync.dma_start(out=ufw_t[:], in_=ufw_d[:])
            w8 = {}
            for nm, d_, cols in (("wxio", wxio_d, 512), ("whio", whio_d, 512),
                                 ("wxu", wxu_d, 256), ("dwxu", dwxu_d, 256),
                                 ("whu", whu_d, 256), ("dwhu", dwhu_d, 256)):
                t = cpool.tile([128, cols], bf16, tag=nm, name=nm)
                nc.sync.dma_start(out=t[:], in_=d_[:])
                w8[nm] = dr(t[:], cols)
            if not f_bias_zero or not iou_bias_zero:
                ones_t = cpool.tile([1, 128], bf16)
                nc.vector.memset(ones_t[:], 1.0)
            if not f_bias_zero:
                ufb_t = cpool.tile([1, H], bf16)
                nc.sync.dma_start(out=ufb_t[:], in_=ufb_d[:])
            if not iou_bias_zero:
                bio_t = cpool.tile([1, 512], bf16)
                nc.sync.dma_start(out=bio_t[:], in_=bio_d[:])
                buu_t = cpool.tile([1, 256], bf16)
                nc.sync.dma_start(out=buu_t[:], in_=buu_d[:])

            state = {}  # rolling per-super tiles + pending cagg work

            def emit_cagg(pend):
                jj, fc_t, s_ap = pend
                b = jj % SUP
                cg_t = psc.tile([128, H], f32, tag="cg", name="cg")
                nc.tensor.matmul(out=cg_t[:], lhsT=s_ap, rhs=fc_t[:],
                                 start=True, stop=True)
                nc.vector.tensor_copy(out=state[("cgs", jj // SUP)][:, b, :],
                                      in_=cg_t[:])

            def emit_apply_piece(s, piece):
                # fuio is section-major [128, sect(i,o,u,f), SUP, 256] so every
                # apply slice below is contiguous (keeps DVE packed fast path).
                # Pieces are spread across blocks so no engine FIFO gets a
                # burst that would stall the PE's software pipeline.
                fuio = state[("fuio", s)]
                if piece == 0:
                    u2 = sp.tile([128, SUP, H], bf16, tag="u2", name="u2")
                    nc.vector.tensor_scalar(
                        out=u2[:], in0=fuio[:, 2, :, :],
                        scalar1=2.0, scalar2=-1.0, op0=MULT, op1=ADD)
                    state[("u2", s)] = u2
                elif piece == 1:
                    iu = sp.tile([128, SUP, H], bf16, tag="iu", name="iu")
                    nc.vector.tensor_tensor(out=iu[:], in0=fuio[:, 0, :, :],
                                            in1=state[("u2", s)][:], op=MULT)
                    state[("iu", s)] = iu
                elif piece == 2:
                    cn = sp.tile([128, SUP, H], bf16, tag="cn", name="cn")
                    nc.vector.tensor_tensor(out=cn[:], in0=state[("iu", s)][:],
                                            in1=state[("cgs", s)][:], op=ADD)
                    state[("cn", s)] = cn
                    nc.sync.dma_start(
                        out=cout_d[:, s * SUP * H:(s + 1) * SUP * H],
                        in_=cn[:])
                elif piece == 3:
                    th = sp.tile([128, SUP, H], bf16, tag="th", name="th")
                    nc.scalar.activation(out=th[:], in_=state[("cn", s)][:],
                                         func=TANH)
                    state[("th", s)] = th
                else:
                    hn = sp.tile([128, SUP, H], bf16, tag="hn", name="hn")
                    nc.vector.tensor_tensor(out=hn[:], in0=fuio[:, 1, :, :],
                                            in1=state[("th", s)][:], op=MULT)
                    nc.sync.dma_start(
                        out=hout_d[:, s * SUP * H:(s + 1) * SUP * H],
                        in_=hn[:])

            pending = []
            for j in range(nblk):
                s, b = j // SUP, j % SUP
                if b == 0:
                    slab_t = iop.tile([128, scols], bf16, tag="slab",
                                      name="slab")
                    nc.sync.dma_start(
                        out=slab_t[:],
                        in_=slab_d[:, s * scols:(s + 1) * scols])
                    state[("slab", s)] = slab_t
                    state[("fuio", s)] = sp4.tile([128, 4, SUP, H], bf16,
                                                  tag="fuio", name="fuio")
                    state[("cgs", s)] = sp4.tile([128, SUP, H], bf16,
                                                 tag="cgs", name="cgs")

                slab_t = state[("slab", s)]
                o = b * BCOLS
                x8 = dr(slab_t[:, o:o + 128], 128)
                dx8 = dr(slab_t[:, o + 128:o + 256], 128)
                ht8 = dr(slab_t[:, o + 256:o + 384], 128)
                dht8 = dr(slab_t[:, o + 384:o + 512], 128)
                blk = psb.tile([128, 4, H], f32, tag="blk", name="blk")
                # io gates (fp8 DoubleRow, K=256 per pass) -> blk[:, 0:2, :]
                nc.tensor.matmul(out=blk[:, 0:2, :], lhsT=x8, rhs=w8["wxio"],
                                 start=True, stop=False, perf_mode=DR)
                # u gate: (x8+dx8)@W8 + x8@dW8 per input -> blk[:, 2, :]
                nc.tensor.matmul(out=blk[:, 2, :], lhsT=x8, rhs=w8["wxu"],
                                 start=True, stop=False, perf_mode=DR)
                nc.tensor.matmul(out=blk[:, 2, :], lhsT=x8, rhs=w8["dwxu"],
                                 start=False, stop=False, perf_mode=DR)
                nc.tensor.matmul(out=blk[:, 2, :], lhsT=dx8, rhs=w8["wxu"],
                                 start=False, stop=False, perf_mode=DR)
                nc.tensor.matmul(out=blk[:, 0:2, :], lhsT=ht8, rhs=w8["whio"],
                                 start=False, stop=iou_bias_zero, perf_mode=DR)
                nc.tensor.matmul(out=blk[:, 2, :], lhsT=ht8, rhs=w8["whu"],
                                 start=False, stop=False, perf_mode=DR)
                nc.tensor.matmul(out=blk[:, 2, :], lhsT=ht8, rhs=w8["dwhu"],
                                 start=False, stop=False, perf_mode=DR)
                nc.tensor.matmul(out=blk[:, 2, :], lhsT=dht8, rhs=w8["whu"],
                                 start=False, stop=iou_bias_zero, perf_mode=DR)
                # f-gate matmuls (bf16) -> blk[:, 3, :]
                for fi in range(2):
                    nc.tensor.matmul(
                        out=blk[:, 3, :],
                        lhsT=slab_t[:, o + 512 + fi * 128:o + 512 + fi * 128 + 128],
                        rhs=ufw_t[:, fi * H:(fi + 1) * H],
                        start=(fi == 0), stop=(fi == 1 and f_bias_zero))
                if not f_bias_zero:
                    nc.tensor.matmul(out=blk[:, 3, :], lhsT=ones_t[:],
                                     rhs=ufb_t[:], start=False, stop=True)
                if not iou_bias_zero:
                    nc.tensor.matmul(out=blk[:, 0:2, :], lhsT=ones_t[:],
                                     rhs=bio_t[:], start=False, stop=True)
                    nc.tensor.matmul(out=blk[:, 2, :], lhsT=ones_t[:],
                                     rhs=buu_t[:], start=False, stop=True)
                # c_agg scatter lagged by two blocks slots in here on the PE,
                # giving sigma(j-2)+fc(j-2) two full PE-blocks of slack
                if len(pending) >= 2:
                    emit_cagg(pending.pop(0))
                # sigmoid over i|o|u|f -> SBUF super tile (section-major)
                nc.scalar.activation(
                    out=state[("fuio", s)][:, :, b, :],
                    in_=blk[:], func=SIG)
                # fc = f * c_child on the (otherwise idle) Pool engine
                fc_t = wp.tile([128, H], bf16, tag="fc", name="fc")
                nc.gpsimd.tensor_tensor(
                    out=fc_t[:], in0=state[("fuio", s)][:, 3, b, :],
                    in1=slab_t[:, o + 768:o + 1024], op=MULT)
                pending.append((j, fc_t, slab_t[:, o + 1024:o + 1152]))
                # apply phase for super s-2, one piece per block so no engine
                # FIFO gets a burst that stalls the PE's software pipeline
                if 1 <= b <= 5 and s >= 2:
                    emit_apply_piece(s - 2, b - 1)

            for p in pending:
                emit_cagg(p)
            # interleave the final two supers' apply pieces so their
            # cross-engine chains overlap instead of running back-to-back
            tail = [nsup - 2, nsup - 1] if nsup >= 2 else [nsup - 1]
            for piece in range(5):
                for s in tail:
                    emit_apply_piece(s, piece)

    nc.compile()
    return nc


def _pack_blocks(deg, nblk):
    """Strict balanced bin packing: nodes (desc degree) into nblk blocks of
    BLK node slots and at most BLK edges. Returns None if impossible."""
    npc = deg.shape[0]
    order = np.argsort(-deg, kind="stable")
    rem_e = np.full(nblk, BLK, np.int64)
    rem_n = np.full(nblk, BLK, np.int64)
    heap = [(-BLK, b) for b in range(nblk)]
    heapq.heapify(heap)
    assign = np.empty(npc, np.int64)
    for nd in order:
        d = int(deg[nd])
        tmp = []
        placed = False
        while heap:
            negre, b = heapq.heappop(heap)
            if rem_n[b] == 0:
                continue
            if d <= rem_e[b]:
                rem_e[b] -= d
                rem_n[b] -= 1
                assign[nd] = b
                placed = True
                if rem_n[b] > 0:
                    heapq.heappush(heap, (-rem_e[b], b))
                break
            tmp.append((negre, b))
        for t in tmp:
            heapq.heappush(heap, t)
        if not placed:
            return None
    blk_sorted = np.argsort(assign[order], kind="stable")
    perm = order[blk_sorted]  # nodes in block-major placement order
    pos = np.empty(npc, np.int64)
    used = np.bincount(assign, minlength=nblk)
    starts = np.concatenate([[0], np.cumsum(used)[:-1]])
    pos[perm] = np.arange(npc) - np.repeat(starts, used)
    return assign, pos, perm, used


def _dr_pack_data(a8, nblk):
    """fp8 [rows = nblk*128, 256 feat] -> DoubleRow slab section
    [128, nblk, 128] viewed as bf16 (feature j*128+p on partition p)."""
    return np.ascontiguousarray(
        a8.reshape(nblk, 128, 2, 128).transpose(3, 0, 2, 1)
    ).reshape(128, nblk, 256).view(nbf16)


def _dr_pack_w(w8):
    """fp8 [K=256, N] -> [128, j2, N] flattened to bf16 [128, N]."""
    n = w8.shape[1]
    return np.ascontiguousarray(
        w8.reshape(2, 128, n).transpose(1, 0, 2)
    ).reshape(128, 2 * n).view(nbf16)


def _prep_inputs(x, h, c, W_iou, U_iou, b_iou, U_f_w, U_f_b,
                 edge_src, edge_dst):
    n = x.shape[0]
    assert n % N_CORES == 0
    npc = n // N_CORES

    x32 = np.asarray(x, np.float32)
    hbf = np.asarray(h, np.float32).astype(nbf16)
    h32 = np.asarray(h, np.float32)
    cbf = np.asarray(c, np.float32).astype(nbf16)
    edge_src = np.asarray(edge_src, np.int64)
    edge_dst = np.asarray(edge_dst, np.int64)
    owner = edge_dst // npc

    nblk = max(-(-npc // BLK), 1)
    nblk = -(-nblk // SUP) * SUP
    packs = None
    while packs is None:
        packs = []
        for kcore in range(N_CORES):
            ldst = edge_dst[owner == kcore] - kcore * npc
            deg = np.bincount(ldst, minlength=npc)
            if deg.max(initial=0) > BLK:
                raise ValueError("node degree exceeds block capacity")
            r = _pack_blocks(deg, nblk)
            if r is None:
                packs = None
                nblk += SUP
                break
            packs.append(r)
    loc = nblk * BLK

    ufwT = np.asarray(U_f_w, np.float32).T          # [in 256, out 256]
    ufw_s = np.ascontiguousarray(
        ufwT.reshape(2, 128, H).transpose(1, 0, 2).reshape(128, 2 * H)
    ).astype(nbf16)
    wiouT = np.asarray(W_iou, np.float32).T          # [256, 768]
    uiouT = np.asarray(U_iou, np.float32).T
    # io weights fp8 [K, 512]; u weights fp8 W8 + residual dW8 [K, 256],
    # pre-doubled for the sigmoid-tanh trick
    wmaps = {}
    for pre, srcw in (("wx", wiouT), ("wh", uiouT)):
        io8 = srcw[:, 0:512].astype(nf8)
        uu = 2.0 * srcw[:, 512:768]
        u8 = uu.astype(nf8)
        du8 = (uu - u8.astype(np.float32)).astype(nf8)
        wmaps[pre + "io"] = _dr_pack_w(io8)
        wmaps[pre + "u"] = _dr_pack_w(u8)
        wmaps["d" + pre + "u"] = _dr_pack_w(du8)
    b_iou_f = np.asarray(b_iou, np.float32).reshape(-1)
    U_f_b_f = np.asarray(U_f_b, np.float32).reshape(-1)
    f_bias_zero = not U_f_b_f.any()
    iou_bias_zero = not b_iou_f.any()

    in_maps = []
    perms = []
    for kcore in range(N_CORES):
        m = owner == kcore
        ldst = edge_dst[m] - kcore * npc
        src = edge_src[m]
        assign, pos, perm, used = packs[kcore]
        blk_id = assign[ldst]
        dstrel = pos[ldst]

        # edge slot layout: 128 slots per block, edges packed in order
        cnt = np.bincount(blk_id, minlength=nblk)
        assert cnt.max(initial=0) <= BLK
        start = np.zeros(nblk, np.int64)
        np.cumsum(cnt[:-1], out=start[1:])
        eorder = np.argsort(blk_id, kind="stable")
        slot_in_blk = np.arange(blk_id.size) - start[blk_id[eorder]]
        flat_pos = blk_id[eorder] * BLK + slot_in_blk
        tot_e = nblk * BLK

        gsrc = src[eorder]
        hch = np.zeros((tot_e, H), nbf16)
        cch = np.zeros((tot_e, H), nbf16)
        hch[flat_pos] = hbf[gsrc]
        cch[flat_pos] = cbf[gsrc]
        Sfl = np.zeros((tot_e, BLK), nbf16)
        Sfl[flat_pos, dstrel[eorder]] = 1.0

        gperm = perm + kcore * npc
        cols = np.concatenate([
            np.arange(s0, s0 + u) for s0, u in zip(range(0, loc, BLK), used)])
        xN = np.zeros((loc, X), np.float32)
        xN[cols] = x32[gperm]

        htild = np.zeros((loc, H), np.float32)
        if src.size:
            slot = blk_id * BLK + dstrel
            so = np.argsort(slot, kind="stable")
            slot_s = slot[so]
            bounds = np.concatenate(
                [[0], np.flatnonzero(np.diff(slot_s)) + 1])
            htild[slot_s[bounds]] = np.add.reduceat(
                h32[src[so]], bounds, axis=0)

        # fp8 + residual for x / htild (shared by io and u gates)
        x8 = xN.astype(nf8)
        dx8 = (xN - x8.astype(np.float32)).astype(nf8)
        ht8 = htild.astype(nf8)
        dht8 = (htild - ht8.astype(np.float32)).astype(nf8)

        # assemble uniform slab: [128, nblk, BCOLS]
        slab = np.empty((128, nblk, BCOLS), nbf16)
        slab[:, :, 0:128] = _dr_pack_data(x8, nblk)
        slab[:, :, 128:256] = _dr_pack_data(dx8, nblk)
        slab[:, :, 256:384] = _dr_pack_data(ht8, nblk)
        slab[:, :, 384:512] = _dr_pack_data(dht8, nblk)
        slab[:, :, 512:768] = hch.reshape(nblk, 128, 2, 128)\
            .transpose(3, 0, 2, 1).reshape(128, nblk, 256)
        slab[:, :, 768:1024] = cch.reshape(nblk, 128, 256)\
            .transpose(1, 0, 2)
        slab[:, :, 1024:1152] = Sfl.reshape(nblk, 128, BLK)\
            .transpose(1, 0, 2)
        slab = slab.reshape(128, nblk * BCOLS)

        im = {"slab": slab, "ufw": ufw_s}
        im.update(wmaps)
        if not f_bias_zero:
            im["ufb"] = U_f_b_f.reshape(1, H).astype(nbf16)
        if not iou_bias_zero:
            im["bio"] = b_iou_f[0:512].reshape(1, 512).astype(nbf16)
            im["buu"] = (2.0 * b_iou_f[512:768]).reshape(1, 256).astype(nbf16)
        in_maps.append(im)
        perms.append((gperm, cols))

    meta = dict(n=n, npc=npc, nblk=nblk, loc=loc,
                f_bias_zero=f_bias_zero, iou_bias_zero=iou_bias_zero,
                perms=perms)
    return in_maps, meta


def kernel(x, h, c, W_iou, U_iou, b_iou, U_f_w, U_f_b, edge_src, edge_dst,
           _trace=False):
    global LAST_EXEC_TIME_NS
    in_maps, meta = _prep_inputs(x, h, c, W_iou, U_iou, b_iou, U_f_w, U_f_b,
                                 edge_src, edge_dst)
    key = (meta["nblk"], meta["f_bias_zero"], meta["iou_bias_zero"])
    nc = _PROGRAM_CACHE.get(key)
    if nc is None:
        nc = _build_program(meta["nblk"], meta["f_bias_zero"],
                            meta["iou_bias_zero"])
        _PROGRAM_CACHE[key] = nc
    if not _trace:
        os.environ.setdefault("BASS_NEVER_TRACE", "1")
    res = run_bass_kernel_spmd(nc, in_maps, list(range(N_CORES)),
                               trace=_trace,
                               trace_cores=[0] if _trace else None)
    if _trace:
        LAST_EXEC_TIME_NS = res.exec_time_ns

    n, nblk, loc = meta["n"], meta["nblk"], meta["loc"]
    h_new = np.empty((n, H), np.float32)
    c_new = np.empty((n, H), np.float32)
    for kcore in range(N_CORES):
        gperm, cols = meta["perms"][kcore]
        hP = res.results[kcore]["houtP"].reshape(128, nblk, H)\
            .transpose(1, 0, 2).reshape(loc, H)
        cP = res.results[kcore]["coutP"].reshape(128, nblk, H)\
            .transpose(1, 0, 2).reshape(loc, H)
        h_new[gperm] = hP[cols].astype(np.float32)
        c_new[gperm] = cP[cols].astype(np.float32)
    return h_new, c_new
